# revision 25
# baseline (speedup 1.0000x reference)
"""Trainium2 Bass kernel for nn_BasicBlock (binary activation + binarized
weight-standardized 3x3 conv + residual + PReLU).

Contract: kernel(**inputs) takes FULL unsharded numpy inputs (keys as in
setup_inputs) and returns the FULL [32, 512, 28, 28] float32 output.
Internally shards the batch dim across 8 NeuronCores (4 images each); the
small conv weight + per-channel vectors are replicated.

Key math facts exploited:
- forward activations are sign(x*beta+b0) in {-1,0,1} and forward weights
  are sf[o]*gain[o]*sign(w_std) with sign in {-1,0,1}, so the conv
  contraction is exact in fp8/bf16 (products are +-1, fp32 PSUM
  accumulation of integers <= 4608); the per-channel scalar alpha*sf*gain
  folds into the epilogue.
- fp8e4 DoubleRow packs two contraction rows per PE cell (2 cin chunks per
  matmul), halving the matmul count.
"""

import numpy as np

import concourse.bass as bass
import concourse.mybir as mybir
import concourse.tile as tile
from concourse import bacc
from concourse.masks import make_identity

# problem constants (hardcoded per harness contract)
N_CORES = 8
N_PER = 4          # images per core (32 / 8)
C = 512            # Cin == Cout
H = W = 28
HP = WP = 30       # zero-padded spatial
TAPS = 9
KFAN = C * TAPS    # 4608 = fan-in per output channel
ALPHA = 0.2
BETA = 1.0
EPS = 1e-5
WS_SCALE = 1.0 / float(np.sqrt(KFAN))  # fan_in**-0.5
NCH = C // 128     # 4 channel chunks of 128
NPAIR = NCH // 2   # 2 DoubleRow pairs of chunks
ROWS_PER_TILE = 14 # output rows per matmul tile
NSPAT = H // ROWS_PER_TILE  # 2 spatial tiles per image
NFREE = ROWS_PER_TILE * WP  # 420: contiguous run incl. 2 pad cols per row
ACT_IMG = 912  # padded 30x30 image (900) + 12 slack: %16==0 for DoubleRow,
               # and covers the last tile's 420-run overhang (482+420=902)

FP32 = mybir.dt.float32
BF16 = mybir.dt.bfloat16
FP8 = mybir.dt.float8e4


def _load_chunked_vec(nc, pool, dram_ap, name):
    """Load a [512] per-channel vector as 4 SBUF tiles of [128, 1]."""
    tiles = []
    for c in range(NCH):
        t = pool.tile([128, 1], FP32, tag=f"{name}_{c}", name=f"{name}_{c}")
        sl = dram_ap[c * 128 : (c + 1) * 128].rearrange("(p o) -> p o", o=1)
        nc.gpsimd.dma_start(out=t, in_=sl)
        tiles.append(t)
    return tiles


def build_program():
    nc = bacc.Bacc(
        "TRN2",
        target_bir_lowering=False,
        debug=False,
        num_devices=1,
        num_swdge_queues=4,
    )
    x_h = nc.declare_dram_parameter("x", [N_PER, C, H, W], FP32, isOutput=False)
    w_h = nc.declare_dram_parameter("conv_weight", [C, C, 3, 3], FP32, isOutput=False)
    gain_h = nc.declare_dram_parameter("gain", [C], FP32, isOutput=False)
    b0_h = nc.declare_dram_parameter("move0_bias", [C], FP32, isOutput=False)
    b1_h = nc.declare_dram_parameter("move1_bias", [C], FP32, isOutput=False)
    pa_h = nc.declare_dram_parameter("prelu_a", [C], FP32, isOutput=False)
    b2_h = nc.declare_dram_parameter("move2_bias", [C], FP32, isOutput=False)
    out_h = nc.declare_dram_parameter("out", [N_PER, C, H, W], FP32, isOutput=True)

    x_ap = x_h[:, :, :, :]
    w_ap = w_h[:, :, :, :]
    out_ap = out_h[:, :, :, :]

    with tile.TileContext(nc) as tc:
        with (
            tc.tile_pool(name="persist", bufs=1) as persist,
            tc.tile_pool(name="scratch", bufs=2) as scratch,
            tc.tile_pool(name="stats", bufs=4) as stats,
            tc.tile_pool(name="epi", bufs=3) as epi,
            tc.tile_pool(name="psum_mm", bufs=5, space="PSUM") as psum_mm,
            tc.tile_pool(name="psum_tr", bufs=3, space="PSUM") as psum_tr,
        ):
            # ---- small per-channel vectors -------------------------------
            # only b0 (xsign bias) is needed early; the rest load after the
            # bulk DMAs are queued so their tiny strided descriptors don't
            # hog DMA engines during the critical w0 load
            b0_c = _load_chunked_vec(nc, persist, b0_h[:], "b0")

            ident = persist.tile([128, 128], BF16, tag="ident")
            make_identity(nc, ident)

            # derived per-channel epilogue constants:
            #   one_minus_a = 1 - prelu_a ; ab1b2 = prelu_a*move1_bias + move2_bias
            def late_vecs():
                gain_c = _load_chunked_vec(nc, persist, gain_h[:], "gain")
                b1_c = _load_chunked_vec(nc, persist, b1_h[:], "b1")
                pa_c = _load_chunked_vec(nc, persist, pa_h[:], "pa")
                b2_c = _load_chunked_vec(nc, persist, b2_h[:], "b2")
                return gain_c, b1_c, pa_c, b2_c

            one_minus_a = []
            ab1b2 = []

            def late_consts(gain_c, b1_c, pa_c, b2_c):
              for c in range(NCH):
                oma = persist.tile([128, 1], FP32, tag=f"oma{c}", name=f"oma{c}")
                nc.vector.tensor_scalar(
                    out=oma, in0=pa_c[c], scalar1=-1.0, scalar2=1.0,
                    op0=mybir.AluOpType.mult, op1=mybir.AluOpType.add,
                )
                one_minus_a.append(oma)
                ab = persist.tile([128, 1], FP32, tag=f"ab1b2{c}", name=f"ab1b2{c}")
                nc.vector.scalar_tensor_tensor(
                    out=ab, in0=b1_c[c], scalar=pa_c[c], in1=b2_c[c],
                    op0=mybir.AluOpType.mult, op1=mybir.AluOpType.add,
                )
                ab1b2.append(ab)

            # ---- weight DMAs: 3 sg-aligned pieces each so chunk 0 lands
            # fast across queues; m>=1 chunks stream in later ------------
            w_flat = w_ap.rearrange("o i a b -> o (i a b)")
            w_tiles = []

            def w_dma(m):
                wt = scratch.tile([128, KFAN], FP32, tag="wtile", name=f"wt{m}")
                for j in range(TAPS):
                    nc.sync.dma_start(
                        out=wt[:, j * 512 : (j + 1) * 512],
                        in_=w_flat[
                            m * 128 : (m + 1) * 128, j * 512 : (j + 1) * 512
                        ],
                    )
                w_tiles.append(wt)


            # ---- activations: sign(x*beta + b0) into padded fp8 ----------
            # act_pair[q] : [128, 2, n, 30, 30] fp8 -- two cin chunks per
            # DoubleRow pair; zeros at spatial border.
            act_img = []  # [q][n] -> [128, 2, ACT_IMG] fp8
            for q in range(NPAIR):
                row = []
                for n in range(N_PER):
                    ap_t = persist.tile(
                        [128, 2, ACT_IMG], FP8, tag=f"act{q}_{n}", name=f"act{q}_{n}"
                    )
                    nc.gpsimd.memset(ap_t, 0.0)
                    row.append(ap_t)
                act_img.append(row)
            xs_tiles = [
                persist.tile(
                    [128, N_PER, H, W], FP32, tag=f"xs{c}", name=f"xs{c}"
                )
                for c in range(NCH)
            ]
            xr = x_ap.rearrange("n c h w -> c n h w")
            # interleaved streaming: w chunk m, then x image m -- w0 gets
            # the queues first, each x image lands just before its convs
            for n in range(N_PER):
                w_dma(n)
                for c in range(NCH):
                    nc.sync.dma_start(
                        out=xs_tiles[c][:, n],
                        in_=xr[c * 128 : (c + 1) * 128, n],
                    )

            def xsign(n, c):
                dst = act_img[c // 2][n][:, c % 2, : HP * WP].rearrange(
                    "p (h w) -> p h w", w=WP
                )[:, 1 : 1 + H, 1 : 1 + W]
                nc.scalar.activation(
                    out=dst,
                    in_=xs_tiles[c][:, n],
                    func=mybir.ActivationFunctionType.Sign,
                    bias=b0_c[c],
                    scale=BETA,
                )

            gain_c, b1_c, pa_c, b2_c = late_vecs()
            late_consts(gain_c, b1_c, pa_c, b2_c)

            # lhsT : [128(cin), tap, pair, half, cout] fp8 DoubleRow weights
            lhsT = persist.tile(
                [128, TAPS, NPAIR, 2, C], FP8, tag="lhsT", name="lhsT"
            )
            alphabar = {}  # per cout chunk [128,1]: alpha*gain*sf

            wsigns = {}
            mvs = {}

            def weight_prep_a(m):
                """stats + binarize -- the critical path to the transposes"""
                wt = w_tiles[m]
                st = stats.tile([128, TAPS, 6], FP32, tag="bnst", name="bnst")
                wt3 = wt.rearrange("p (a b) -> p a b", b=512)
                for sg in range(TAPS):
                    nc.vector.bn_stats(out=st[:, sg, :], in_=wt3[:, sg, :])
                mv = stats.tile([128, 2], FP32, tag="bnagg", name="bnagg")
                nc.vector.bn_aggr(out=mv, in_=st)

                negmean = stats.tile([128, 1], FP32, tag="negmean", name="negmean")
                nc.vector.tensor_scalar_mul(out=negmean, in0=mv[:, 0:1], scalar1=-1.0)

                # sign(w - mean) -> bf16, split per cin block for finer
                # ACT interleaving
                ws = scratch.tile([128, KFAN], BF16, tag="wsign", name="wsign")
                for b in range(NCH):
                    nc.scalar.activation(
                        out=ws[:, b * 1152 : (b + 1) * 1152],
                        in_=wt[:, b * 1152 : (b + 1) * 1152],
                        func=mybir.ActivationFunctionType.Sign,
                        bias=negmean,
                    )
                wsigns[m] = ws
                mvs[m] = (mv, negmean)

            def weight_prep_b(m):
                """transpose to [cin, (pair, half), cout] per tap; 4 cin-block
                transposes share one PSUM bank -> single batched DVE cast"""
                ws3 = wsigns[m].rearrange("p (i t) -> p i t", t=TAPS)
                for t in range(TAPS):
                    ps = psum_tr.tile([128, NCH * 128], BF16, tag="ptr", name="ptr")
                    for b in range(NCH):
                        nc.tensor.transpose(
                            ps[:, b * 128 : (b + 1) * 128],
                            ws3[:, b * 128 : (b + 1) * 128, t],
                            ident,
                        )
                    nc.vector.tensor_copy(
                        out=lhsT[:, t, :, :, m * 128 : (m + 1) * 128], in_=ps
                    )

            def weight_prep_c(m):
                """1/(std+eps), sum|w-mean| -> alphabar; off critical path"""
                wt = w_tiles[m]
                mv, negmean = mvs[m]
                stdeps = stats.tile([128, 1], FP32, tag="stdeps", name="stdeps")
                nc.scalar.activation(
                    out=stdeps, in_=mv[:, 1:2], func=mybir.ActivationFunctionType.Sqrt
                )
                nc.vector.tensor_scalar_add(out=stdeps, in0=stdeps, scalar1=EPS)
                inv = stats.tile([128, 1], FP32, tag="inv", name="inv")
                nc.vector.reciprocal(out=inv, in_=stdeps)

                sumabs = stats.tile([128, NCH], FP32, tag="sumabs", name="sumabs")
                for b in range(NCH):
                    nc.scalar.activation(
                        out=wt[:, b * 1152 : (b + 1) * 1152],
                        in_=wt[:, b * 1152 : (b + 1) * 1152],
                        func=mybir.ActivationFunctionType.Abs,
                        bias=negmean,
                        accum_out=sumabs[:, b : b + 1],
                    )
                sumabs1 = stats.tile([128, 1], FP32, tag="sumabs1", name="sumabs1")
                nc.vector.tensor_reduce(
                    out=sumabs1, in_=sumabs, axis=mybir.AxisListType.X,
                    op=mybir.AluOpType.add,
                )

                ab = persist.tile(
                    [128, 1], FP32, tag=f"alphabar{m}", name=f"alphabar{m}"
                )
                nc.vector.tensor_tensor(
                    out=ab, in0=sumabs1, in1=inv, op=mybir.AluOpType.mult
                )
                nc.vector.tensor_tensor(
                    out=ab, in0=ab, in1=gain_c[m], op=mybir.AluOpType.mult
                )
                nc.vector.tensor_scalar_mul(
                    out=ab, in0=ab, scalar1=ALPHA * WS_SCALE / KFAN
                )
                alphabar[m] = ab

            def conv_block(m, mid_cb=None):
                for n in range(N_PER):
                    if n == 2 and mid_cb is not None:
                        mid_cb()
                    for h2 in range(NSPAT):
                        y0 = h2 * ROWS_PER_TILE
                        acc = psum_mm.tile(
                            [128, NFREE], FP32, tag="acc", name="acc"
                        )
                        i = 0
                        for q in range(NPAIR):
                            for t in range(TAPS):
                                dy, dx = t // 3, t % 3
                                base = (y0 + dy) * WP + dx
                                rhs = act_img[q][n][:, :, base : base + NFREE]
                                nc.tensor.matmul(
                                    acc,
                                    lhsT[:, t, q, :, m * 128 : (m + 1) * 128],
                                    rhs,
                                    start=(i == 0),
                                    stop=(i == NPAIR * TAPS - 1),
                                    perf_mode=mybir.MatmulPerfMode.DoubleRow,
                                )
                                i += 1
                        accv = acc.rearrange("p (h w) -> p h w", w=WP)[
                            :, :, 0:W
                        ]

                        res = xs_tiles[m][:, n, y0 : y0 + ROWS_PER_TILE, :]
                        # z = acc*alphabar + residual   (prelu input minus b1)
                        z = epi.tile(
                            [128, ROWS_PER_TILE, W], FP32, tag="z", name="z"
                        )
                        nc.vector.scalar_tensor_tensor(
                            out=z, in0=accv, scalar=alphabar[m], in1=res,
                            op0=mybir.AluOpType.mult, op1=mybir.AluOpType.add,
                        )
                        # r = relu(z + b1) on ACT
                        r = epi.tile(
                            [128, ROWS_PER_TILE, W], FP32, tag="r", name="r"
                        )
                        nc.scalar.activation(
                            out=r, in_=z,
                            func=mybir.ActivationFunctionType.Relu,
                            bias=b1_c[m],
                        )
                        # zz = a*z + (a*b1 + b2) ; out = (1-a)*r + zz
                        zz = epi.tile(
                            [128, ROWS_PER_TILE, W], FP32, tag="zz", name="zz"
                        )
                        nc.scalar.activation(
                            out=zz, in_=z,
                            func=mybir.ActivationFunctionType.Identity,
                            scale=pa_c[m], bias=ab1b2[m],
                        )
                        nc.vector.scalar_tensor_tensor(
                            out=zz, in0=r, scalar=one_minus_a[m], in1=zz,
                            op0=mybir.AluOpType.mult, op1=mybir.AluOpType.add,
                        )
                        nc.sync.dma_start(
                            out=out_ap[
                                n, m * 128 : (m + 1) * 128,
                                y0 : y0 + ROWS_PER_TILE, :,
                            ],
                            in_=zz,
                        )

            # interleave: PE stays busy on conv(m) while ACT/DVE prep chunk
            # m+1. wpA(0)/wpB(0) outrank the xsigns on ACT so the weight
            # critical path isn't queued behind 13us of activation signs.
            weight_prep_a(0)
            weight_prep_b(0)
            for n in range(N_PER):
                for c in range(NCH):
                    xsign(n, c)
            weight_prep_c(0)
            for m in range(NCH):
                if m + 1 < NCH:
                    weight_prep_a(m + 1)
                conv_block(m)
                if m + 1 < NCH:
                    weight_prep_b(m + 1)
                    weight_prep_c(m + 1)

    nc.finalize()
    return nc


_NC_CACHE = None


def _get_program():
    global _NC_CACHE
    if _NC_CACHE is None:
        _NC_CACHE = build_program()
    return _NC_CACHE


def kernel(**inputs):
    from concourse.bass_utils import run_bass_kernel_spmd

    x = np.ascontiguousarray(np.asarray(inputs["x"], dtype=np.float32))
    shared = {
        name: np.ascontiguousarray(np.asarray(inputs[name], dtype=np.float32))
        for name in (
            "conv_weight", "gain", "move0_bias", "move1_bias", "prelu_a",
            "move2_bias",
        )
    }
    nc = _get_program()
    in_maps = [
        {"x": x[i * N_PER : (i + 1) * N_PER], **shared} for i in range(N_CORES)
    ]
    res = run_bass_kernel_spmd(nc, in_maps, core_ids=list(range(N_CORES)))
    return np.concatenate([r["out"] for r in res.results], axis=0)
